# revision 9
# baseline (speedup 1.0000x reference)
"""Segment-mean + tiny classifier (ALLonBert post-encoder) on 8 TRN2 cores.

Data-parallel over batch: each of the 8 cores handles 2 of the 16 batch
rows (flattened to [8192, 1024]). hidden_states streams in as 16 wide
[128, 4x1024] f32 tiles (2 MiB DMAs). Per 128-token chunk, a one-hot
[token, segment] matrix is built on GpSimd from host-precomputed segment
ids (is_equal against an iota row), the f32 tile is cast to bf16 (split
between VectorE and ScalarE), and TensorE accumulates M_T.T @ hidden
into PSUM [128 segs, 1024] across all 64 chunks. The 2-class classifier
is a DVE mul+reduce against a partition-replicated W, scaled by 1/count
with the bias added, then a tiny DMA out.
"""

import sys

if "/opt/trn_rl_repo" not in sys.path:
    sys.path.insert(0, "/opt/trn_rl_repo")

import numpy as np

B, S, H = 16, 4096, 1024
NSEG = 64
SEP_ID = 102
NCORES = 8
RPC = B // NCORES          # batch rows per core
T = RPC * S                # tokens per core
NCHUNK = T // 128          # 128-token tiles per core
MSEG = RPC * NSEG          # output segments per core (= 128)
GROUP = 4                  # chunks per DMA
NG = NCHUNK // GROUP

_CACHE: dict = {}


def _build():
    if "nc" in _CACHE:
        return _CACHE["nc"]
    from concourse import bacc, tile, mybir
    import concourse.bass as bass

    f32 = mybir.dt.float32
    bf16 = mybir.dt.bfloat16
    A = mybir.AluOpType

    nc = bacc.Bacc(None, target_bir_lowering=False, debug=False)
    hidden = nc.declare_dram_parameter("hidden", [T, H], f32, isOutput=False)
    seg = nc.declare_dram_parameter("seg", [128, NCHUNK], f32, isOutput=False)
    w2 = nc.declare_dram_parameter("w2", [128, 2 * H], f32, isOutput=False)
    invb = nc.declare_dram_parameter("invb", [128, 3], f32, isOutput=False)
    out = nc.declare_dram_parameter("out", [128, 2], f32, isOutput=True)

    hv = hidden[:].rearrange("(g a p) h -> g p a h", g=NG, a=GROUP, p=128)

    with tile.TileContext(nc) as tc:
        with (
            tc.tile_pool(name="const", bufs=1) as cpool,
            tc.tile_pool(name="hid", bufs=3) as hpool,
            tc.tile_pool(name="mt", bufs=16) as mpool,
            tc.tile_pool(name="fin", bufs=2) as fpool,
            tc.tile_pool(name="psum", bufs=1, space=bass.MemorySpace.PSUM) as ppool,
        ):
            iota_t = cpool.tile([128, 128], f32)
            nc.gpsimd.iota(
                iota_t[:],
                pattern=[[1, 128]],
                base=0,
                channel_multiplier=0,
                allow_small_or_imprecise_dtypes=True,
            )
            seg_t = cpool.tile([128, NCHUNK], f32)
            nc.sync.dma_start(seg_t[:], seg[:])
            w_t = cpool.tile([128, 2 * H], f32)
            nc.sync.dma_start(w_t[:], w2[:])
            invb_t = cpool.tile([128, 3], f32)
            nc.sync.dma_start(invb_t[:], invb[:])

            ps0 = ppool.tile([128, 512], f32)
            ps1 = ppool.tile([128, 512], f32)

            for g in range(NG):
                hid_t = hpool.tile([128, GROUP * H], f32)
                nc.sync.dma_start(
                    hid_t[:].rearrange("p (a h) -> p a h", a=GROUP), hv[g]
                )
                hb_t = hpool.tile([128, GROUP * H], bf16, tag="hb")
                hid_3d = hid_t[:].rearrange("p (a h) -> p a h", a=GROUP)
                hb_3d = hb_t[:].rearrange("p (a h) -> p a h", a=GROUP)
                nc.vector.tensor_copy(hb_3d[:, :, 0:512], hid_3d[:, :, 0:512])
                nc.scalar.copy(hb_3d[:, :, 512:H], hid_3d[:, :, 512:H])
                mts = []
                for a in range(GROUP):
                    c = g * GROUP + a
                    mt_t = mpool.tile([128, 128], bf16)
                    nc.gpsimd.tensor_scalar(
                        mt_t[:], iota_t[:], seg_t[:, c : c + 1], None, op0=A.is_equal
                    )
                    mts.append(mt_t)
                for a in range(GROUP):
                    c = g * GROUP + a
                    first, last = c == 0, c == NCHUNK - 1
                    lo = a * H
                    nc.tensor.matmul(
                        ps0[:], mts[a][:], hb_t[:, lo : lo + 512],
                        start=first, stop=last,
                    )
                    nc.tensor.matmul(
                        ps1[:], mts[a][:], hb_t[:, lo + 512 : lo + H],
                        start=first, stop=last,
                    )

            # red layout: cols 0-1 = bank0 (cls0, cls1), cols 2-3 = bank1
            red = cpool.tile([128, 4], f32)
            for bank, ps in enumerate((ps0, ps1)):
                for cls in range(2):
                    scr = fpool.tile([128, 512], f32, tag="scr")
                    nc.vector.tensor_tensor(
                        scr[:], ps[:], w_t[:, cls * H + bank * 512 :][:, 0:512],
                        op=A.mult,
                    )
                    nc.vector.tensor_reduce(
                        red[:, 2 * bank + cls : 2 * bank + cls + 1],
                        scr[:],
                        axis=mybir.AxisListType.X,
                        op=A.add,
                    )
            summed = cpool.tile([128, 2], f32)
            nc.vector.tensor_tensor(summed[:], red[:, 0:2], red[:, 2:4], op=A.add)
            scaled = cpool.tile([128, 2], f32)
            nc.vector.tensor_scalar(
                scaled[:], summed[:], invb_t[:, 0:1], None, op0=A.mult
            )
            logit = cpool.tile([128, 2], f32)
            nc.vector.tensor_tensor(logit[:], scaled[:], invb_t[:, 1:3], op=A.add)
            nc.sync.dma_start(out[:], logit[:])

    nc.compile()
    _CACHE["nc"] = nc
    return nc


def _host_prep(hidden_states, classifier_w, classifier_b, input_ids):
    ids = np.asarray(input_ids)
    sep = ids == SEP_ID
    seg = np.cumsum(sep, axis=1) - sep.astype(np.int64)          # [B, S]
    pos = np.arange(S)
    num_seps = sep.sum(axis=1, keepdims=True)
    valid = (~sep) & (pos[None, :] >= 1) & (seg < num_seps)      # [B, S]

    counts = np.zeros((B, NSEG), np.float32)
    for b in range(B):
        cb = np.bincount(seg[b][valid[b]], minlength=NSEG)[:NSEG]
        counts[b] = cb
    inv = 1.0 / np.maximum(counts, 1.0)                          # [B, NSEG]

    # flat per-core segment id of each token, -1 when the token is dropped
    flat = np.where(valid, seg, -1).astype(np.int64)             # [B, S]

    W = np.asarray(classifier_w, dtype=np.float32)
    bvec = np.asarray(classifier_b, dtype=np.float32)
    w2 = np.ascontiguousarray(
        np.broadcast_to(W.reshape(1, 2 * H), (128, 2 * H)).astype(np.float32)
    )

    hs = np.asarray(hidden_states, dtype=np.float32)
    in_maps = []
    for i in range(NCORES):
        rows = slice(RPC * i, RPC * (i + 1))
        fl = flat[rows].copy()                                   # [RPC, S]
        for r in range(RPC):
            m = fl[r] >= 0
            fl[r][m] += r * NSEG
        flt = fl.reshape(T)                                      # [T]
        seg_param = np.ascontiguousarray(
            flt.reshape(NCHUNK, 128).T.astype(np.float32)
        )                                                        # [128, NCHUNK]
        invc = inv[rows].reshape(MSEG)                           # [128]
        invb = np.ascontiguousarray(
            np.stack(
                [invc, np.full(MSEG, bvec[0]), np.full(MSEG, bvec[1])], axis=1
            ).astype(np.float32)
        )                                                        # [128, 3]
        in_maps.append(
            {
                "hidden": np.ascontiguousarray(hs[rows].reshape(T, H)),
                "seg": seg_param,
                "w2": w2,
                "invb": invb,
            }
        )
    return in_maps


def kernel(hidden_states, classifier_w, classifier_b, input_ids, n_segs):
    from concourse.bass_utils import run_bass_kernel_spmd

    nc = _build()
    in_maps = _host_prep(hidden_states, classifier_w, classifier_b, input_ids)
    res = run_bass_kernel_spmd(nc, in_maps, core_ids=list(range(NCORES)))
    outs = [res.results[i]["out"].reshape(RPC, NSEG, 2) for i in range(NCORES)]
    return np.concatenate(outs, axis=0).astype(np.float32)


# revision 10
# speedup vs baseline: 1.5522x; 1.5522x over previous
"""Segment-mean + tiny classifier (ALLonBert post-encoder) on 8 TRN2 cores.

Data-parallel over batch: each of the 8 cores handles 2 of the 16 batch
rows (flattened to [8192, 1024]). hidden_states streams in as 16 wide
[128, 4x1024] f32 tiles (2 MiB DMAs). Per 128-token chunk, a one-hot
[token, segment] matrix is built on GpSimd from host-precomputed segment
ids (is_equal against an iota row), the f32 tile is cast to bf16 (split
between VectorE and ScalarE), and TensorE accumulates M_T.T @ hidden
into PSUM [128 segs, 1024] across all 64 chunks. The 2-class classifier
is a DVE mul+reduce against a partition-replicated W, scaled by 1/count
with the bias added, then a tiny DMA out.
"""

import sys

if "/opt/trn_rl_repo" not in sys.path:
    sys.path.insert(0, "/opt/trn_rl_repo")

import numpy as np

B, S, H = 16, 4096, 1024
NSEG = 64
SEP_ID = 102
NCORES = 8
RPC = B // NCORES          # batch rows per core
T = RPC * S                # tokens per core
NCHUNK = T // 128          # 128-token tiles per core
MSEG = RPC * NSEG          # output segments per core (= 128)
GROUP = 4                  # chunks per DMA
NG = NCHUNK // GROUP

_CACHE: dict = {}


def _build():
    if "nc" in _CACHE:
        return _CACHE["nc"]
    from concourse import bacc, tile, mybir
    import concourse.bass as bass

    f32 = mybir.dt.float32
    bf16 = mybir.dt.bfloat16
    A = mybir.AluOpType

    nc = bacc.Bacc(None, target_bir_lowering=False, debug=False)
    hidden = nc.declare_dram_parameter("hidden", [T, H], f32, isOutput=False)
    seg = nc.declare_dram_parameter("seg", [128, NCHUNK], f32, isOutput=False)
    w2 = nc.declare_dram_parameter("w2", [128, 2 * H], f32, isOutput=False)
    invb = nc.declare_dram_parameter("invb", [128, 3], f32, isOutput=False)
    out = nc.declare_dram_parameter("out", [128, 2], f32, isOutput=True)

    hv = hidden[:].rearrange("(g a p) h -> g p a h", g=NG, a=GROUP, p=128)

    with tile.TileContext(nc) as tc:
        with (
            tc.tile_pool(name="const", bufs=1) as cpool,
            tc.tile_pool(name="hid", bufs=3) as hpool,
            tc.tile_pool(name="mt", bufs=16) as mpool,
            tc.tile_pool(name="fin", bufs=2) as fpool,
            tc.tile_pool(name="psum", bufs=1, space=bass.MemorySpace.PSUM) as ppool,
        ):
            iota_t = cpool.tile([128, 128], f32)
            nc.gpsimd.iota(
                iota_t[:],
                pattern=[[1, 128]],
                base=0,
                channel_multiplier=0,
                allow_small_or_imprecise_dtypes=True,
            )
            seg_t = cpool.tile([128, NCHUNK], f32)
            nc.sync.dma_start(seg_t[:], seg[:])
            w_t = cpool.tile([128, 2 * H], f32)
            nc.sync.dma_start(w_t[:], w2[:])
            invb_t = cpool.tile([128, 3], f32)
            nc.sync.dma_start(invb_t[:], invb[:])

            ps0 = ppool.tile([128, 512], f32)
            ps1 = ppool.tile([128, 512], f32)

            for g in range(NG):
                hid_t = hpool.tile([128, GROUP * H], f32)
                nc.sync.dma_start(
                    hid_t[:].rearrange("p (a h) -> p a h", a=GROUP), hv[g]
                )
                hb_t = hpool.tile([128, GROUP * H], bf16, tag="hb")
                mts = []
                for a in range(GROUP):
                    c = g * GROUP + a
                    lo = a * H
                    nc.vector.tensor_copy(
                        hb_t[:, lo : lo + 512], hid_t[:, lo : lo + 512]
                    )
                    nc.scalar.copy(
                        hb_t[:, lo + 512 : lo + H], hid_t[:, lo + 512 : lo + H]
                    )
                    mt_t = mpool.tile([128, 128], bf16)
                    nc.vector.tensor_scalar(
                        mt_t[:], iota_t[:], seg_t[:, c : c + 1], None, op0=A.is_equal
                    )
                    mts.append(mt_t)
                for a in range(GROUP):
                    c = g * GROUP + a
                    first, last = c == 0, c == NCHUNK - 1
                    lo = a * H
                    nc.tensor.matmul(
                        ps0[:], mts[a][:], hb_t[:, lo : lo + 512],
                        start=first, stop=last,
                    )
                    nc.tensor.matmul(
                        ps1[:], mts[a][:], hb_t[:, lo + 512 : lo + H],
                        start=first, stop=last,
                    )

            # red layout: cols 0-1 = bank0 (cls0, cls1), cols 2-3 = bank1
            red = cpool.tile([128, 4], f32)
            for bank, ps in enumerate((ps0, ps1)):
                for cls in range(2):
                    scr = fpool.tile([128, 512], f32, tag="scr")
                    nc.vector.tensor_tensor(
                        scr[:], ps[:], w_t[:, cls * H + bank * 512 :][:, 0:512],
                        op=A.mult,
                    )
                    nc.vector.tensor_reduce(
                        red[:, 2 * bank + cls : 2 * bank + cls + 1],
                        scr[:],
                        axis=mybir.AxisListType.X,
                        op=A.add,
                    )
            summed = cpool.tile([128, 2], f32)
            nc.vector.tensor_tensor(summed[:], red[:, 0:2], red[:, 2:4], op=A.add)
            scaled = cpool.tile([128, 2], f32)
            nc.vector.tensor_scalar(
                scaled[:], summed[:], invb_t[:, 0:1], None, op0=A.mult
            )
            logit = cpool.tile([128, 2], f32)
            nc.vector.tensor_tensor(logit[:], scaled[:], invb_t[:, 1:3], op=A.add)
            nc.sync.dma_start(out[:], logit[:])

    nc.compile()
    _CACHE["nc"] = nc
    return nc


def _host_prep(hidden_states, classifier_w, classifier_b, input_ids):
    ids = np.asarray(input_ids)
    sep = ids == SEP_ID
    seg = np.cumsum(sep, axis=1) - sep.astype(np.int64)          # [B, S]
    pos = np.arange(S)
    num_seps = sep.sum(axis=1, keepdims=True)
    valid = (~sep) & (pos[None, :] >= 1) & (seg < num_seps)      # [B, S]

    counts = np.zeros((B, NSEG), np.float32)
    for b in range(B):
        cb = np.bincount(seg[b][valid[b]], minlength=NSEG)[:NSEG]
        counts[b] = cb
    inv = 1.0 / np.maximum(counts, 1.0)                          # [B, NSEG]

    # flat per-core segment id of each token, -1 when the token is dropped
    flat = np.where(valid, seg, -1).astype(np.int64)             # [B, S]

    W = np.asarray(classifier_w, dtype=np.float32)
    bvec = np.asarray(classifier_b, dtype=np.float32)
    w2 = np.ascontiguousarray(
        np.broadcast_to(W.reshape(1, 2 * H), (128, 2 * H)).astype(np.float32)
    )

    hs = np.asarray(hidden_states, dtype=np.float32)
    in_maps = []
    for i in range(NCORES):
        rows = slice(RPC * i, RPC * (i + 1))
        fl = flat[rows].copy()                                   # [RPC, S]
        for r in range(RPC):
            m = fl[r] >= 0
            fl[r][m] += r * NSEG
        flt = fl.reshape(T)                                      # [T]
        seg_param = np.ascontiguousarray(
            flt.reshape(NCHUNK, 128).T.astype(np.float32)
        )                                                        # [128, NCHUNK]
        invc = inv[rows].reshape(MSEG)                           # [128]
        invb = np.ascontiguousarray(
            np.stack(
                [invc, np.full(MSEG, bvec[0]), np.full(MSEG, bvec[1])], axis=1
            ).astype(np.float32)
        )                                                        # [128, 3]
        in_maps.append(
            {
                "hidden": np.ascontiguousarray(hs[rows].reshape(T, H)),
                "seg": seg_param,
                "w2": w2,
                "invb": invb,
            }
        )
    return in_maps


def kernel(hidden_states, classifier_w, classifier_b, input_ids, n_segs):
    from concourse.bass_utils import run_bass_kernel_spmd

    nc = _build()
    in_maps = _host_prep(hidden_states, classifier_w, classifier_b, input_ids)
    res = run_bass_kernel_spmd(nc, in_maps, core_ids=list(range(NCORES)))
    outs = [res.results[i]["out"].reshape(RPC, NSEG, 2) for i in range(NCORES)]
    return np.concatenate(outs, axis=0).astype(np.float32)
